# revision 12
# baseline (speedup 1.0000x reference)
"""Trainium2 Bass kernel for nn_AMTCL_77867757077077 (AMTCL triplet-center loss).

Key insight: the reference's [B,B] distance matrix dist[i,j] depends on j only
through targets[j], so it has just C=100 distinct columns:

    dist[i,j] = S[i, t_j],  S[i,k] = sqrt(q[k] - 2*(x @ u.T)[i,k] + (x^2 @ v.T)[i,k])

with v = 2^w, u = v*c, q[k] = sum_d v[k,d]*c[k,d]^2.  Then
    dist_ap[i] = S[i, t_i]
    dist_an[i] = min_{k != t_i, k present} S[i,k]
    per_sample = ap + relu(cc - an),  cc = centers_dist[t_i]
so the O(B^2 D) problem collapses to two [B,C] matmuls (O(B C D)).

All center-side prep is host-side (it is O(C*D) / O(C^2*D) constant work the
original computes as a detached numpy constant anyway): v=2^w, u2=-2*v*c,
q[k], the closest-center distance cd[k], the absent-class penalty, and
cc_row[i]=cd[t_i].  x^2 ships as fp8 so the device runs no element-wise prep
ahead of the matmuls.

The whole contraction is THREE fp8 DoubleRowSwInterleave matmuls (2 k-tiles
per instruction at 0.5 cycles/row): (u,k01)@x, (v,k01)@x2, and the k2 tails
of u and v paired into one dual-row instruction by interleaving the rhs
chunks [x_k0 x_k1 x2_k0 x2_k1 x_k2 x2_k2] in a single [128,6,BL] tile.
Weights are pre-interleaved host-side in the SwInterleave layout (pairwise,
reversed column order, M padded to 128) so LDWEIGHTS is a plain load; q+pen
rides the same DMA as 4 trailing f32 bytes per partition (bitcast for the
sqrt bias).

Sharding: data-parallel over batch rows, 8 cores x 512 rows; center tensors
replicated.  Each core emits its partial sum/B; the host adds the 8 scalars
(the "all-reduce" of the hint, done on 8 floats host-side).

Device-side structure per core (everything in the matmul's native
[class k, row i] layout):
    s_ps[k,i]   = PSUM accum of the 3 dual-row matmuls = d2 - q
    st          = sqrt(s_ps + (q+pen)[k])        (ACT, per-partition bias)
    ohT[k,i]    = (t_bcast - k == 0)             (one DVE op; t pre-broadcast)
    sbig[:C]    = st + 1e9*ohT  (bf16); row C holds cc (DMA'd from host)
    per 128-col tile: PE-transpose [C+1,128] -> [128,C+1]; row-min of
                  cols :C = an; col C - an = cc - an
    ap          = free-axis accumulators of st*ohT (DVE, one per half)
    loss_part   = (sum ap + sum relu(cc-an)) / B via K=1 matmuls
"""

import numpy as np

NUM_CORES = 8
B = 4096
D = 384
C = 100
BL = B // NUM_CORES  # 512 rows per core
P = 128
NT = BL // P         # 4 row tiles per core
KD = D // P          # 3 contraction chunks

_CACHE = {}


def _build_nc():
    import concourse.bass as bass
    import concourse.bass_isa as bass_isa
    import concourse.bacc as bacc
    import concourse.tile as tile
    from concourse import mybir
    from concourse.masks import make_identity
    from contextlib import ExitStack

    f32 = mybir.dt.float32
    bf16 = mybir.dt.bfloat16
    fp8 = mybir.dt.float8e4

    nc = bacc.Bacc(
        "TRN2",
        target_bir_lowering=False,
        debug=False,
        enable_asserts=False,
        num_devices=NUM_CORES,
    )

    # TWO partition-major byte tensors, one per HWDGE stream (DMAs on the
    # same issuing engine serialize; sync and scalar overlap at ~195GB/s
    # each).  A carries everything the first dual-row matmul needs; B the
    # rest, in need order.
    # A: [0:768] weights fp8 u_sw|v_sw|uvk2_sw (SwInterleave layout) |
    #    [768:772] qpen f32 | [772:1796] xx pair0 = (x_k0|x_k1)[p]
    # B: [0:1024] xx pair1 = (x2_k0|x2_k1)[p] | [1024:2048] xx pair2 =
    #    (x_k2|x2_k2)[p] | [2048:3072] t broadcast bf16
    AW = 6 * P + 4 + 2 * BL
    BW = 4 * BL + 2 * BL
    biga_ext = nc.dram_tensor("biga", [P, AW], mybir.dt.uint8, kind="ExternalInput").ap()
    bigb_ext = nc.dram_tensor("bigb", [P, BW], mybir.dt.uint8, kind="ExternalInput").ap()
    cc_ext = nc.dram_tensor("cc", [1, BL], bf16, kind="ExternalInput").ap()
    out_ext = nc.dram_tensor("out", [1, 1], f32, kind="ExternalOutput").ap()

    with tile.TileContext(nc) as tc, ExitStack() as ctx:
        singles = ctx.enter_context(tc.tile_pool(name="singles", bufs=1))
        ps_big = ctx.enter_context(tc.tile_pool(name="psbig", bufs=1, space="PSUM"))

        # ---- gpsimd constants first (keeps its queue clear of the DMAs)
        iota_col = singles.tile([P, 1], f32)   # value = partition index
        nc.gpsimd.iota(
            iota_col,
            pattern=[[0, 1]],
            base=0,
            channel_multiplier=1,
            allow_small_or_imprecise_dtypes=True,
        )

        # ---- input DMAs: A on sync, B on scalar, cc trailing on sync
        biga = singles.tile([P, AW], mybir.dt.uint8)
        nc.sync.dma_start(biga, biga_ext)
        bigb = singles.tile([P, BW], mybir.dt.uint8)
        nc.scalar.dma_start(bigb, bigb_ext)
        sbig = singles.tile([C, BL], bf16)           # st + 1e9*ohT
        cc_row = singles.tile([1, BL], bf16)         # cd[t_i]
        nc.sync.dma_start(cc_row, cc_ext)
        tb = bigb[:, 4 * BL :].bitcast(bf16)              # [P, BL]

        # ---- other constants
        invB_col = singles.tile([P, 1], bf16)
        nc.vector.memset(invB_col, 1.0 / B)   # also the sqrt-warmup operand
        # Sqrt table warmup: a dependency-free ACT so the Sqrt act-table load
        # runs during the DMA wait instead of gating the real sqrts
        sqrt_warm = singles.tile([1, 1], f32)
        nc.scalar.activation(
            sqrt_warm, invB_col[0:1, :], mybir.ActivationFunctionType.Sqrt
        )

        # ohT[k,i] = ((tb[k,i] - k) == 0), exact 0/1 in bf16
        ohT = singles.tile([P, BL], bf16)
        nc.vector.tensor_scalar(
            out=ohT, in0=tb, scalar1=iota_col[:, :], scalar2=0.0,
            op0=mybir.AluOpType.subtract, op1=mybir.AluOpType.is_equal,
        )

        # ---- the whole contraction: 3 dual-row fp8 matmuls into one PSUM bank
        s_ps = ps_big.tile([P, BL], f32)
        DRSW = mybir.MatmulPerfMode.DoubleRowSwInterleave
        rhs_aps = [
            biga[:, 6 * P + 4 :],
            bigb[:, 0 : 2 * BL],
            bigb[:, 2 * BL : 4 * BL],
        ]
        for j in range(3):
            nc.tensor.matmul(
                s_ps,
                lhsT=biga[:, j * 2 * P : (j + 1) * 2 * P].bitcast(fp8),
                rhs=rhs_aps[j].bitcast(fp8).rearrange("p (k i) -> p k i", k=2),
                start=(j == 0), stop=(j == 2), perf_mode=DRSW,
            )

        qpen_v = biga[:, 6 * P : 6 * P + 4].bitcast(f32)  # [P,1] f32 view

        # ---- st = sqrt(d2) ; sbig = st + 1e9*ohT ; ap accum per half
        st_sb = singles.tile([C, BL], bf16)
        fin3 = singles.tile([1, 5], f32)    # [relu_sum | apA | apB | an junk..]
        ap2 = singles.tile([C, 2], f32)
        w1x = singles.tile([C, BL], bf16)
        H = BL // 2

        for h in range(2):
            sl = slice(h * H, (h + 1) * H)
            nc.scalar.activation(
                st_sb[:, sl], s_ps[:C, sl],
                mybir.ActivationFunctionType.Sqrt, bias=qpen_v[:C, :],
            )
            nc.vector.scalar_tensor_tensor(
                out=sbig[:, sl], in0=ohT[:C, sl], scalar=-1e9, in1=st_sb[:, sl],
                op0=mybir.AluOpType.mult, op1=mybir.AluOpType.subtract,
            )
            nc.vector.scalar_tensor_tensor(
                out=w1x[:, sl], in0=st_sb[:, sl], scalar=1.0, in1=ohT[:C, sl],
                op0=mybir.AluOpType.mult, op1=mybir.AluOpType.mult,
                accum_out=ap2[:, h : h + 1],
            )

        # an for all samples in one shot: cross-partition max of the
        # negated masked distances (cross-lane reduce has no min op):
        # sbig = -st - 1e9*oh, so max over classes = -an, replicated to all
        # partitions by the all-reduce
        negmax_all = singles.tile([C, BL], f32)
        nc.gpsimd.partition_all_reduce(
            negmax_all, sbig, channels=C, reduce_op=bass_isa.ReduceOp.max
        )
        # dcol = cc - an = cc + negmax ; relu_sum = sum(max(dcol, 0))
        dcol_row = singles.tile([1, BL], f32)
        nc.vector.scalar_tensor_tensor(
            out=dcol_row, in0=negmax_all[0:1, :], scalar=1.0, in1=cc_row,
            op0=mybir.AluOpType.mult, op1=mybir.AluOpType.add,
        )
        junk_row = singles.tile([1, BL], bf16)
        nc.vector.tensor_scalar(
            out=junk_row, in0=dcol_row, scalar1=0.0, scalar2=None,
            op0=mybir.AluOpType.max, op1=mybir.AluOpType.add,
            accum_out=fin3[:, 0:1],
        )
        # ap class-sums -> two scalars, same partition-axis trick
        nc.gpsimd.tensor_reduce(
            fin3[:, 1:3], ap2, axis=mybir.AxisListType.C, op=mybir.AluOpType.add
        )
        # loss_part = (relu_sum + apA + apB) / B
        junk3 = singles.tile([1, 3], f32)
        out_sb = singles.tile([1, 1], f32)
        nc.vector.tensor_scalar(
            out=junk3, in0=fin3[:, 0:3], scalar1=1.0 / B, scalar2=None,
            op0=mybir.AluOpType.mult, op1=mybir.AluOpType.add,
            accum_out=out_sb,
        )
        nc.sync.dma_start(out_ext, out_sb)

    nc.compile()
    return nc


def _get_nc():
    if "nc" not in _CACHE:
        _CACHE["nc"] = _build_nc()
    return _CACHE["nc"]


def make_in_maps(inputs, targets, centers, centers_weights):
    import ml_dtypes

    x = np.asarray(inputs, np.float32)
    t = np.asarray(targets).astype(np.int64)
    c = np.asarray(centers, np.float32)
    w = np.asarray(centers_weights, np.float32)
    assert x.shape == (B, D) and c.shape == (C, D) and w.shape == (C, D)

    fp8 = ml_dtypes.float8_e4m3
    bf16 = ml_dtypes.bfloat16

    xt_all = np.ascontiguousarray(x.astype(fp8).T)        # [D, B]
    x2t_all = np.ascontiguousarray((x * x).astype(fp8).T)  # [D, B]

    v = np.exp2(w)                                 # [C, D]
    u2 = -2.0 * v * c

    def pad_chunks(m_t):
        # [D, C] transposed weights, zero-padded to M=128, split into k-chunks
        m_p = np.zeros((D, P), fp8)
        m_p[:, :C] = m_t.astype(fp8)
        return m_p.reshape(KD, P, P)

    def ilv(a, b):
        # SwInterleave weight layout: pairwise interleave in reversed col order
        sw = np.empty((P, 2 * P), fp8)
        sw[:, 0::2] = a[:, ::-1]
        sw[:, 1::2] = b[:, ::-1]
        return sw

    uk = pad_chunks(u2.T)
    vk = pad_chunks(v.T)
    q = np.einsum("kd,kd->k", v, c * c).astype(np.float32)  # [C]
    present = np.bincount(t, minlength=C) > 0
    pen = np.where(present, 0.0, 1e12).astype(np.float32)
    qpen = np.zeros(P, np.float32)
    qpen[:C] = q + pen
    qpen_bytes = qpen.astype("<f4").reshape(P, 1).view(np.uint8)  # [P,4]

    ctr = np.ascontiguousarray(
        np.concatenate(
            [
                ilv(uk[0], uk[1]).view(np.uint8),
                ilv(vk[0], vk[1]).view(np.uint8),
                ilv(uk[2], vk[2]).view(np.uint8),
                qpen_bytes,
            ],
            axis=1,
        )
    )                                              # [P, 6P+4] raw bytes

    # closest-center distance (the reference computes this as a detached
    # numpy constant): second-smallest of the row-sorted weighted distances
    diff = c[:, None, :] - c[None, :, :]           # [C, C, D]
    d2c = np.einsum("cd,ced->ce", v, diff * diff)
    dc = np.sqrt(d2c)
    dc.sort(axis=1)
    cd = dc[:, 1].astype(np.float32)               # [C]
    cc_all = cd[t]                                 # [B]

    t_bf = t.astype(bf16)                          # targets < 100: exact in bf16

    in_maps = []
    for i in range(NUM_CORES):
        sl = slice(i * BL, (i + 1) * BL)
        xk = xt_all[:, sl].reshape(KD, P, BL)
        x2k = x2t_all[:, sl].reshape(KD, P, BL)
        tb = np.broadcast_to(t_bf[sl][None, :], (P, BL))
        p0 = np.stack([xk[0], xk[1]], axis=1).reshape(P, 2 * BL)
        p1 = np.stack([x2k[0], x2k[1]], axis=1).reshape(P, 2 * BL)
        p2 = np.stack([xk[2], x2k[2]], axis=1).reshape(P, 2 * BL)
        biga = np.concatenate([ctr, p0.view(np.uint8)], axis=1)
        bigb = np.concatenate(
            [p1.view(np.uint8), p2.view(np.uint8),
             tb.view(np.uint8).reshape(P, 2 * BL)],
            axis=1,
        )
        in_maps.append(
            {
                "biga": np.ascontiguousarray(biga),
                "bigb": np.ascontiguousarray(bigb),
                "cc": np.ascontiguousarray(cc_all[sl].astype(bf16).reshape(1, BL)),
            }
        )
    return in_maps


def run(inputs, targets, centers, centers_weights, trace=False):
    """Build+run the SPMD kernel; returns (loss_scalar, BassKernelResults)."""
    from concourse import bass_utils

    nc = _get_nc()
    in_maps = make_in_maps(inputs, targets, centers, centers_weights)
    res = None
    for attempt in range(3):
        try:
            res = bass_utils.run_bass_kernel_spmd(
                nc, in_maps, core_ids=list(range(NUM_CORES)), trace=trace
            )
            break
        except Exception:
            # A previously-crashed session can leave the device in a transient
            # "unrecoverable" state that clears on the next attempt.
            if attempt == 2:
                raise
    loss = np.float32(0.0)
    for r in res.results:
        loss += np.float32(r["out"][0, 0])
    return np.array(loss, dtype=np.float32), res


def kernel(inputs, targets, epoch_number=None, centers=None, centers_weights=None):
    loss, _ = run(inputs, targets, centers, centers_weights, trace=False)
    return loss


# revision 13
# speedup vs baseline: 1.4100x; 1.4100x over previous
"""Trainium2 Bass kernel for nn_AMTCL_77867757077077 (AMTCL triplet-center loss).

Key insight: the reference's [B,B] distance matrix dist[i,j] depends on j only
through targets[j], so it has just C=100 distinct columns:

    dist[i,j] = S[i, t_j],  S[i,k] = sqrt(q[k] - 2*(x @ u.T)[i,k] + (x^2 @ v.T)[i,k])

with v = 2^w, u = v*c, q[k] = sum_d v[k,d]*c[k,d]^2.  Then
    dist_ap[i] = S[i, t_i]
    dist_an[i] = min_{k != t_i, k present} S[i,k]
    per_sample = ap + relu(cc - an),  cc = centers_dist[t_i]
so the O(B^2 D) problem collapses to two [B,C] matmuls (O(B C D)).

All center-side prep is host-side (it is O(C*D) / O(C^2*D) constant work the
original computes as a detached numpy constant anyway): v=2^w, u2=-2*v*c,
q[k], the closest-center distance cd[k], the absent-class penalty, and
cc_row[i]=cd[t_i].  x^2 ships as fp8 so the device runs no element-wise prep
ahead of the matmuls.

The whole contraction is THREE fp8 DoubleRowSwInterleave matmuls (2 k-tiles
per instruction at 0.5 cycles/row): (u,k01)@x, (v,k01)@x2, and the k2 tails
of u and v paired into one dual-row instruction by interleaving the rhs
chunks [x_k0 x_k1 x2_k0 x2_k1 x_k2 x2_k2] in a single [128,6,BL] tile.
Weights are pre-interleaved host-side in the SwInterleave layout (pairwise,
reversed column order, M padded to 128) so LDWEIGHTS is a plain load; q+pen
rides the same DMA as 4 trailing f32 bytes per partition (bitcast for the
sqrt bias).

Sharding: data-parallel over batch rows, 8 cores x 512 rows; center tensors
replicated.  Each core emits its partial sum/B; the host adds the 8 scalars
(the "all-reduce" of the hint, done on 8 floats host-side).

Device-side structure per core (everything in the matmul's native
[class k, row i] layout):
    s_ps[k,i]   = PSUM accum of the 3 dual-row matmuls = d2 - q
    st          = sqrt(s_ps + (q+pen)[k])        (ACT, per-partition bias)
    ohT[k,i]    = (t_bcast - k == 0)             (one DVE op; t pre-broadcast)
    sbig[:C]    = st + 1e9*ohT  (bf16); row C holds cc (DMA'd from host)
    per 128-col tile: PE-transpose [C+1,128] -> [128,C+1]; row-min of
                  cols :C = an; col C - an = cc - an
    ap          = free-axis accumulators of st*ohT (DVE, one per half)
    loss_part   = (sum ap + sum relu(cc-an)) / B via K=1 matmuls
"""

import numpy as np

NUM_CORES = 8
B = 4096
D = 384
C = 100
BL = B // NUM_CORES  # 512 rows per core
P = 128
NT = BL // P         # 4 row tiles per core
KD = D // P          # 3 contraction chunks

_CACHE = {}


def _build_nc():
    import concourse.bass as bass
    import concourse.bass_isa as bass_isa
    import concourse.bacc as bacc
    import concourse.tile as tile
    from concourse import mybir
    from concourse.masks import make_identity
    from contextlib import ExitStack

    f32 = mybir.dt.float32
    bf16 = mybir.dt.bfloat16
    fp8 = mybir.dt.float8e4

    nc = bacc.Bacc(
        "TRN2",
        target_bir_lowering=False,
        debug=False,
        enable_asserts=False,
        num_devices=NUM_CORES,
    )

    # TWO partition-major byte tensors, one per HWDGE stream (DMAs on the
    # same issuing engine serialize; sync and scalar overlap at ~195GB/s
    # each).  A carries everything the first dual-row matmul needs; B the
    # rest, in need order.
    # A: [0:768] weights fp8 u_sw|v_sw|uvk2_sw (SwInterleave layout) |
    #    [768:772] qpen f32 | [772:1796] xx pair0 = (x_k0|x_k1)[p]
    # B: [0:1024] xx pair1 = (x2_k0|x2_k1)[p] | [1024:2048] xx pair2 =
    #    (x_k2|x2_k2)[p] | [2048:3072] t broadcast bf16
    AW = 6 * P + 4 + 2 * BL
    BW = 4 * BL + 2 * BL
    biga_ext = nc.dram_tensor("biga", [P, AW], mybir.dt.uint8, kind="ExternalInput").ap()
    bigb_ext = nc.dram_tensor("bigb", [P, BW], mybir.dt.uint8, kind="ExternalInput").ap()
    cc_ext = nc.dram_tensor("cc", [1, BL], bf16, kind="ExternalInput").ap()
    out_ext = nc.dram_tensor("out", [1, 1], f32, kind="ExternalOutput").ap()

    with tile.TileContext(nc) as tc, ExitStack() as ctx:
        singles = ctx.enter_context(tc.tile_pool(name="singles", bufs=1))
        ps_big = ctx.enter_context(tc.tile_pool(name="psbig", bufs=1, space="PSUM"))
        ps_tr = ctx.enter_context(tc.tile_pool(name="pstr", bufs=2, space="PSUM"))
        ps_misc = ctx.enter_context(tc.tile_pool(name="psmisc", bufs=1, space="PSUM"))

        # ---- gpsimd constants first (keeps its queue clear of the DMAs)
        ident_bf = singles.tile([P, P], bf16)
        make_identity(nc, ident_bf)
        iota_col = singles.tile([P, 1], f32)   # value = partition index
        nc.gpsimd.iota(
            iota_col,
            pattern=[[0, 1]],
            base=0,
            channel_multiplier=1,
            allow_small_or_imprecise_dtypes=True,
        )

        # ---- input DMAs: A on sync, B on scalar, cc trailing on sync
        biga = singles.tile([P, AW], mybir.dt.uint8)
        nc.sync.dma_start(biga, biga_ext)
        bigb = singles.tile([P, BW], mybir.dt.uint8)
        nc.scalar.dma_start(bigb, bigb_ext)
        sbig = singles.tile([C + 1, BL], bf16)       # st + 1e9*ohT; row C = cc
        nc.sync.dma_start(sbig[C : C + 1, :], cc_ext)
        tb = bigb[:, 4 * BL :].bitcast(bf16)              # [P, BL]

        # ---- other constants
        invB_col = singles.tile([P, 1], bf16)
        nc.vector.memset(invB_col, 1.0 / B)   # also the sqrt-warmup operand
        # Sqrt table warmup: a dependency-free ACT so the Sqrt act-table load
        # runs during the DMA wait instead of gating the real sqrts
        sqrt_warm = singles.tile([1, 1], f32)
        nc.scalar.activation(
            sqrt_warm, invB_col[0:1, :], mybir.ActivationFunctionType.Sqrt
        )

        # ohT[k,i] = ((tb[k,i] - k) == 0), exact 0/1 in bf16
        ohT = singles.tile([P, BL], bf16)
        nc.vector.tensor_scalar(
            out=ohT, in0=tb, scalar1=iota_col[:, :], scalar2=0.0,
            op0=mybir.AluOpType.subtract, op1=mybir.AluOpType.is_equal,
        )

        # ---- the whole contraction: 3 dual-row fp8 matmuls into one PSUM bank
        s_ps = ps_big.tile([P, BL], f32)
        DRSW = mybir.MatmulPerfMode.DoubleRowSwInterleave
        rhs_aps = [
            biga[:, 6 * P + 4 :],
            bigb[:, 0 : 2 * BL],
            bigb[:, 2 * BL : 4 * BL],
        ]
        for j in range(3):
            nc.tensor.matmul(
                s_ps,
                lhsT=biga[:, j * 2 * P : (j + 1) * 2 * P].bitcast(fp8),
                rhs=rhs_aps[j].bitcast(fp8).rearrange("p (k i) -> p k i", k=2),
                start=(j == 0), stop=(j == 2), perf_mode=DRSW,
            )

        qpen_v = biga[:, 6 * P : 6 * P + 4].bitcast(f32)  # [P,1] f32 view

        # ---- st = sqrt(d2) ; sbig = st + 1e9*ohT ; ap accum per half
        st_sb = singles.tile([C, BL], bf16)
        fin3 = singles.tile([1, 5], f32)    # [relu_sum | apA | apB | an junk..]
        ap2 = singles.tile([C, 2], f32)
        w1x = singles.tile([C, BL], bf16)
        H = BL // 2

        for h in range(2):
            sl = slice(h * H, (h + 1) * H)
            nc.scalar.activation(
                st_sb[:, sl], s_ps[:C, sl],
                mybir.ActivationFunctionType.Sqrt, bias=qpen_v[:C, :],
            )
            nc.vector.scalar_tensor_tensor(
                out=sbig[:C, sl], in0=ohT[:C, sl], scalar=1e9, in1=st_sb[:, sl],
                op0=mybir.AluOpType.mult, op1=mybir.AluOpType.add,
            )
            nc.vector.scalar_tensor_tensor(
                out=w1x[:, sl], in0=st_sb[:, sl], scalar=1.0, in1=ohT[:C, sl],
                op0=mybir.AluOpType.mult, op1=mybir.AluOpType.mult,
                accum_out=ap2[:, h : h + 1],
            )

        # per 128-col tile: PE transpose of [C+1,128] (classes + cc row),
        # DVE row-min of class cols = an, col C - an = cc - an
        mnc = singles.tile([P, NT], bf16)
        dcol = singles.tile([P, NT], bf16)
        for t in range(NT):
            sl = slice(t * P, (t + 1) * P)
            st_ps = ps_tr.tile([P, C + 1], bf16)
            nc.tensor.transpose(
                st_ps, sbig[: C + 1, sl], ident_bf[: C + 1, : C + 1]
            )
            nc.vector.tensor_reduce(
                mnc[:, t : t + 1], st_ps[:, :C], axis=mybir.AxisListType.X,
                op=mybir.AluOpType.min,
            )
            nc.vector.tensor_sub(
                dcol[:, t : t + 1], st_ps[:, C : C + 1], mnc[:, t : t + 1]
            )

        # sum relu(cc - an) + sum ap, then / B via K=1 matmuls (all bf16)
        junkc = singles.tile([P, NT], bf16)
        relu_part = singles.tile([P, 1], bf16)
        nc.vector.tensor_scalar(
            out=junkc, in0=dcol, scalar1=0.0, scalar2=None,
            op0=mybir.AluOpType.max, op1=mybir.AluOpType.add,
            accum_out=relu_part,
        )
        ap_bf = singles.tile([C, 2], bf16)
        nc.vector.tensor_copy(ap_bf, ap2)
        fin_ps = ps_misc.tile([1, 1], f32, tag="misc")
        nc.tensor.matmul(fin_ps, lhsT=invB_col, rhs=relu_part, start=True, stop=False)
        nc.tensor.matmul(
            fin_ps, lhsT=invB_col[:C, :], rhs=ap_bf[:, 0:1], start=False, stop=False
        )
        nc.tensor.matmul(
            fin_ps, lhsT=invB_col[:C, :], rhs=ap_bf[:, 1:2], start=False, stop=True
        )
        out_sb = singles.tile([1, 1], f32)
        nc.vector.tensor_copy(out_sb, fin_ps)
        nc.sync.dma_start(out_ext, out_sb)

    nc.compile()
    return nc


def _get_nc():
    if "nc" not in _CACHE:
        _CACHE["nc"] = _build_nc()
    return _CACHE["nc"]


def make_in_maps(inputs, targets, centers, centers_weights):
    import ml_dtypes

    x = np.asarray(inputs, np.float32)
    t = np.asarray(targets).astype(np.int64)
    c = np.asarray(centers, np.float32)
    w = np.asarray(centers_weights, np.float32)
    assert x.shape == (B, D) and c.shape == (C, D) and w.shape == (C, D)

    fp8 = ml_dtypes.float8_e4m3
    bf16 = ml_dtypes.bfloat16

    xt_all = np.ascontiguousarray(x.astype(fp8).T)        # [D, B]
    x2t_all = np.ascontiguousarray((x * x).astype(fp8).T)  # [D, B]

    v = np.exp2(w)                                 # [C, D]
    u2 = -2.0 * v * c

    def pad_chunks(m_t):
        # [D, C] transposed weights, zero-padded to M=128, split into k-chunks
        m_p = np.zeros((D, P), fp8)
        m_p[:, :C] = m_t.astype(fp8)
        return m_p.reshape(KD, P, P)

    def ilv(a, b):
        # SwInterleave weight layout: pairwise interleave in reversed col order
        sw = np.empty((P, 2 * P), fp8)
        sw[:, 0::2] = a[:, ::-1]
        sw[:, 1::2] = b[:, ::-1]
        return sw

    uk = pad_chunks(u2.T)
    vk = pad_chunks(v.T)
    q = np.einsum("kd,kd->k", v, c * c).astype(np.float32)  # [C]
    present = np.bincount(t, minlength=C) > 0
    pen = np.where(present, 0.0, 1e12).astype(np.float32)
    qpen = np.zeros(P, np.float32)
    qpen[:C] = q + pen
    qpen_bytes = qpen.astype("<f4").reshape(P, 1).view(np.uint8)  # [P,4]

    ctr = np.ascontiguousarray(
        np.concatenate(
            [
                ilv(uk[0], uk[1]).view(np.uint8),
                ilv(vk[0], vk[1]).view(np.uint8),
                ilv(uk[2], vk[2]).view(np.uint8),
                qpen_bytes,
            ],
            axis=1,
        )
    )                                              # [P, 6P+4] raw bytes

    # closest-center distance (the reference computes this as a detached
    # numpy constant): second-smallest of the row-sorted weighted distances
    diff = c[:, None, :] - c[None, :, :]           # [C, C, D]
    d2c = np.einsum("cd,ced->ce", v, diff * diff)
    dc = np.sqrt(d2c)
    dc.sort(axis=1)
    cd = dc[:, 1].astype(np.float32)               # [C]
    cc_all = cd[t]                                 # [B]

    t_bf = t.astype(bf16)                          # targets < 100: exact in bf16

    in_maps = []
    for i in range(NUM_CORES):
        sl = slice(i * BL, (i + 1) * BL)
        xk = xt_all[:, sl].reshape(KD, P, BL)
        x2k = x2t_all[:, sl].reshape(KD, P, BL)
        tb = np.broadcast_to(t_bf[sl][None, :], (P, BL))
        p0 = np.stack([xk[0], xk[1]], axis=1).reshape(P, 2 * BL)
        p1 = np.stack([x2k[0], x2k[1]], axis=1).reshape(P, 2 * BL)
        p2 = np.stack([xk[2], x2k[2]], axis=1).reshape(P, 2 * BL)
        biga = np.concatenate([ctr, p0.view(np.uint8)], axis=1)
        bigb = np.concatenate(
            [p1.view(np.uint8), p2.view(np.uint8),
             tb.view(np.uint8).reshape(P, 2 * BL)],
            axis=1,
        )
        in_maps.append(
            {
                "biga": np.ascontiguousarray(biga),
                "bigb": np.ascontiguousarray(bigb),
                "cc": np.ascontiguousarray(cc_all[sl].astype(bf16).reshape(1, BL)),
            }
        )
    return in_maps


def run(inputs, targets, centers, centers_weights, trace=False):
    """Build+run the SPMD kernel; returns (loss_scalar, BassKernelResults)."""
    from concourse import bass_utils

    nc = _get_nc()
    in_maps = make_in_maps(inputs, targets, centers, centers_weights)
    res = None
    for attempt in range(3):
        try:
            res = bass_utils.run_bass_kernel_spmd(
                nc, in_maps, core_ids=list(range(NUM_CORES)), trace=trace
            )
            break
        except Exception:
            # A previously-crashed session can leave the device in a transient
            # "unrecoverable" state that clears on the next attempt.
            if attempt == 2:
                raise
    loss = np.float32(0.0)
    for r in res.results:
        loss += np.float32(r["out"][0, 0])
    return np.array(loss, dtype=np.float32), res


def kernel(inputs, targets, epoch_number=None, centers=None, centers_weights=None):
    loss, _ = run(inputs, targets, centers, centers_weights, trace=False)
    return loss
